# revision 6
# baseline (speedup 1.0000x reference)
"""Distributed multi-head attention kernel for 8 TRN2 NeuronCores.

Problem: x(4,2048,1024) -> qkv proj (w_qkv 3072x1024) -> 16-head attention
(head_dim 64) -> out proj (w_out 1024x1024 + b_out).

Sharding: head-parallel. Core c owns heads {2c, 2c+1}: it computes Q/K/V for
those heads over all 8192 tokens, runs attention, then an AllToAll converts
the head-sharded attention output into a token-sharded layout (1024 tokens
per core, all 16 heads) for the output projection. No all-reduce needed.

Per-core dataflow:
  1. QKV: Y^T = Wc @ X^T on PE (float32r, full rate). Q^T/K^T kept f32 in
     SBUF ([128 = 2 heads x 64 dims, 8192 tokens]); V written token-major
     bf16 with a ones-column appended (65 wide) per 128-token chunk.
  2. Attention per (batch, head, 512-wide q-tile): S^T tiles [128 k, 512 q]
     via PE (f32r, scale folded into Q); exp on ScalarE (PSUM->SBUF bf16);
     P.V via PE bf16 with lhsT = V-chunk [128, 65] -- the 65th output row
     accumulates the softmax denominators. Normalize with reciprocal +
     partition_broadcast + multiply; store O^T bf16 to the A2A buffer.
  3. AllToAll (2MB bf16) over token chunks.
  4. Out proj: out = O^T.T @ w_out^T + b_out per 128-token tile (bf16 PE,
     f32 accumulate), bias added on VectorE, DMA to the core's output slice.
"""

import numpy as np
import ml_dtypes

import concourse.bass as bass
import concourse.mybir as mybir
import concourse.tile as tile
from concourse import bacc, bass_utils
from concourse.tile import add_dep_helper

FP32 = mybir.dt.float32
FP32R = mybir.dt.float32r
BF16 = mybir.dt.bfloat16
AF = mybir.ActivationFunctionType

N_CORES = 8
B, NTOK, D = 4, 2048, 1024
T = B * NTOK  # 8192 tokens total
NH, HD = 16, 64
HL = NH // N_CORES  # 2 heads per core
SCALE = float(HD) ** -0.5  # 0.125
TN = 512  # token tile for QKV / q tile for attention
NT = T // TN  # 16
KC = D // 128  # 8 contraction chunks for projections
KT = NTOK // 128  # 16 k-chunks per batch in attention
TPC = T // N_CORES  # 1024 tokens per core after A2A
WCOLS = 3 * HL * HD  # 384 qkv output dims per core


def build_nc(debug=False):
    nc = bacc.Bacc(
        "TRN2", target_bir_lowering=False, debug=False, num_devices=N_CORES
    )
    xt = nc.dram_tensor("xt", [D, T], BF16, kind="ExternalInput").ap()
    wt = nc.dram_tensor("wt", [D, WCOLS], BF16, kind="ExternalInput").ap()
    wo = nc.dram_tensor("wo", [D, D], BF16, kind="ExternalInput").ap()
    bias = nc.dram_tensor("bias", [1, D], FP32, kind="ExternalInput").ap()
    out = nc.dram_tensor("out", [TPC, D], FP32, kind="ExternalOutput").ap()
    if debug:
        qdump = nc.dram_tensor("qdump", [128, T], FP32, kind="ExternalOutput").ap()
        kdump = nc.dram_tensor("kdump", [128, T], FP32, kind="ExternalOutput").ap()
        vdump = nc.dram_tensor(
            "vdump", [128, (T // 128) * HL * 65], FP32, kind="ExternalOutput"
        ).ap()
        adump = nc.dram_tensor(
            "adump", [N_CORES, HL * HD, TPC], FP32, kind="ExternalOutput"
        ).ap()

    with tile.TileContext(nc) as tc:
        with (
            tc.tile_pool(name="const", bufs=1) as const,
            tc.tile_pool(name="xin", bufs=10) as xin,
            tc.tile_pool(name="probs", bufs=3) as probs,
            tc.tile_pool(name="norm", bufs=2) as norm,
            tc.tile_pool(name="ot", bufs=3) as otp,
            tc.tile_pool(name="fin", bufs=2) as fin,
            tc.tile_pool(name="psum", bufs=2, space="PSUM") as psum,
            tc.tile_pool(name="dram", bufs=1, space="DRAM") as dram,
        ):
            # ---- persistent SBUF state ----
            w_sb = const.tile([128, KC * WCOLS], BF16)
            nc.sync.dma_start(
                w_sb[:].rearrange("p (kc j) -> p kc j", kc=KC),
                wt.rearrange("(kc p) j -> p kc j", p=128),
            )
            wo_sb = const.tile([128, KC * D], BF16)
            nc.sync.dma_start(
                wo_sb[:].rearrange("p (kc n) -> p kc n", kc=KC),
                wo.rearrange("(kc p) n -> p kc n", p=128),
            )
            b_row = const.tile([1, D], FP32)
            nc.sync.dma_start(b_row[:], bias[:])
            bias_sb = const.tile([128, D], FP32)
            nc.gpsimd.partition_broadcast(bias_sb[:], b_row[:])

            q_sb = const.tile([128, T], BF16)  # [2 heads x 64, tokens], scaled
            k_sb = const.tile([128, T], BF16)
            # V token-major: [128 tok-in-chunk, (global chunk, head) x 65]
            v_sb = const.tile([128, (T // 128) * HL * 65], BF16)
            v3 = v_sb[:].rearrange("p (blk e) -> p blk e", e=65)
            nc.vector.memset(v3[:, :, 64:65], 1.0)

            a2a_in = dram.tile([N_CORES, HL * HD, TPC], BF16)
            a2a_out = dram.tile([N_CORES, HL * HD, TPC], BF16)

            # ---- phase 1: QKV projection ----
            for t in range(NT):
                y_ps = psum.tile([128, 1536], FP32, tag="big3", name="y_ps")
                xts = []
                for kc in range(KC):
                    x_t = xin.tile([128, TN], BF16, tag="xt", name="x_t")
                    nc.sync.dma_start(
                        x_t[:], xt[kc * 128 : (kc + 1) * 128, t * TN : (t + 1) * TN]
                    )
                    xts.append(x_t)
                for kc in range(KC):
                    st, sp = kc == 0, kc == KC - 1
                    for m in range(2):  # Q then K, transposed layout
                        nc.tensor.matmul(
                            y_ps[:, m * 512 : (m + 1) * 512],
                            lhsT=w_sb[
                                :, kc * WCOLS + m * 128 : kc * WCOLS + (m + 1) * 128
                            ],
                            rhs=xts[kc][:],
                            start=st,
                            stop=sp,
                        )
                # V natural layout: 4 token subtiles share one PSUM bank, and
                # start=True clears has_written flags bank-wide -- each
                # subtile's accumulation group must fully finish before the
                # next one starts, so chain explicit ordering deps.
                prev = None
                for s in range(4):
                    for kc in range(KC):
                        st, sp = kc == 0, kc == KC - 1
                        mm = nc.tensor.matmul(
                            y_ps[:, 1024 + s * 128 : 1024 + (s + 1) * 128],
                            lhsT=xts[kc][:, s * 128 : (s + 1) * 128],
                            rhs=w_sb[
                                :, kc * WCOLS + 256 : kc * WCOLS + WCOLS
                            ],
                            start=st,
                            stop=sp,
                        )
                        if prev is not None:
                            add_dep_helper(
                                mm.ins, prev.ins, sync=False,
                                reason="bank flag-clear order",
                            )
                        prev = mm
                nc.scalar.activation(
                    q_sb[:, t * TN : (t + 1) * TN], y_ps[:, 0:512], AF.Copy,
                    scale=SCALE,
                )
                nc.scalar.activation(
                    k_sb[:, t * TN : (t + 1) * TN], y_ps[:, 512:1024], AF.Copy
                )
                # psum V region: [128 tok, (s, h, d)] -> v_sb blocks (t*4+s, h)
                nc.scalar.activation(
                    v3[:, (t * 4) * HL : (t * 4 + 4) * HL, 0:64],
                    y_ps[:, 1024:1536].rearrange("p (s hd) -> p s hd", s=4)
                    .rearrange("p s (h d) -> p (s h) d", h=HL),
                    AF.Copy,
                )

            # ---- phase 2: attention ----
            # (kc, h) slot pairs packed into [128, 1536] PSUM tiles, 3 slots
            # each; heads interleaved so PE row-packing (base partition 0/64)
            # can run both heads' S matmuls concurrently.
            for b in range(B):
                for qt in range(NTOK // TN):
                    q_off = b * NTOK + qt * TN
                    ch = q_off // TPC
                    col = q_off % TPC
                    pv = [
                        psum.tile([128, 512], FP32, tag="pv", name=f"pv{h}")
                        for h in range(HL)
                    ]
                    slot_list = [(kc, h) for kc in range(KT) for h in range(HL)]
                    for g0 in range(0, len(slot_list), 3):
                        group = slot_list[g0 : g0 + 3]
                        width = len(group) * 512
                        s_t = psum.tile([128, 1536], FP32, tag="big3", name="s_t")
                        for i, (kc, h) in enumerate(group):
                            nc.tensor.matmul(
                                s_t[:, i * 512 : (i + 1) * 512],
                                lhsT=k_sb[
                                    h * 64 : (h + 1) * 64,
                                    b * NTOK + kc * 128 : b * NTOK + (kc + 1) * 128,
                                ],
                                rhs=q_sb[
                                    h * 64 : (h + 1) * 64, q_off : q_off + TN
                                ],
                                start=True,
                                stop=True,
                            )
                        p_t = probs.tile([128, 1536], BF16, tag="p", name="p_t")
                        nc.scalar.activation(
                            p_t[:, 0:width], s_t[:, 0:width], AF.Exp
                        )
                        for i, (kc, h) in enumerate(group):
                            gc = b * KT + kc
                            nc.tensor.matmul(
                                pv[h][0:65, :],
                                lhsT=v3[:, gc * HL + h, :],
                                rhs=p_t[:, i * 512 : (i + 1) * 512],
                                start=(kc == 0),
                                stop=(kc == KT - 1),
                            )
                    for h in range(HL):
                        recip = norm.tile([1, 512], FP32, tag="recip", name="recip")
                        nc.vector.reciprocal(recip[:], pv[h][64:65, :])
                        bc = norm.tile([64, 512], FP32, tag="bc", name="bc")
                        nc.gpsimd.partition_broadcast(bc[:], recip[:])
                        o_t = otp.tile([64, 512], BF16, tag="o", name="o_t")
                        nc.vector.tensor_mul(o_t[:], pv[h][0:64, :], bc[:])
                        nc.sync.dma_start(
                            a2a_in[ch, h * 64 : (h + 1) * 64, col : col + TN],
                            o_t[:],
                        )

            if debug:
                dbf = fin.tile([128, 1024], FP32, tag="outt", name="dbf")
                for t in range(NT):
                    d1 = fin.tile([128, TN], FP32, tag="outt", name="d1")
                    nc.vector.tensor_copy(d1[:], q_sb[:, t * TN : (t + 1) * TN])
                    nc.sync.dma_start(qdump[:, t * TN : (t + 1) * TN], d1[:])
                    d2 = fin.tile([128, TN], FP32, tag="outt", name="d2")
                    nc.vector.tensor_copy(d2[:], k_sb[:, t * TN : (t + 1) * TN])
                    nc.sync.dma_start(kdump[:, t * TN : (t + 1) * TN], d2[:])
                nv = (T // 128) * HL * 65
                for j in range(0, nv, 1024):
                    wdt = min(1024, nv - j)
                    d3 = fin.tile([128, 1024], FP32, tag="outt", name="d3")
                    nc.vector.tensor_copy(d3[:, 0:wdt], v_sb[:, j : j + wdt])
                    nc.sync.dma_start(vdump[:, j : j + wdt], d3[:, 0:wdt])
                for i in range(N_CORES):
                    d4 = fin.tile([128, TPC], BF16, tag="outt", name="d4")
                    nc.sync.dma_start(d4[:], a2a_in[i, :, :])
                    d5 = fin.tile([128, TPC], FP32, tag="outt", name="d5")
                    nc.vector.tensor_copy(d5[:], d4[:])
                    nc.sync.dma_start(adump[i, :, :], d5[:])

            # ---- phase 3: AllToAll over token chunks ----
            nc.gpsimd.collective_compute(
                "AllToAll",
                mybir.AluOpType.bypass,
                replica_groups=[list(range(N_CORES))],
                ins=[a2a_in.opt()],
                outs=[a2a_out.opt()],
            )

            # ---- phase 4: output projection ----
            o_sb = const.tile([128, N_CORES * TPC], BF16)
            for i in range(N_CORES):
                nc.sync.dma_start(
                    o_sb[:, i * TPC : (i + 1) * TPC], a2a_out[i, :, :]
                )
            for m in range(TPC // 128):
                o_ps = psum.tile([128, 1536], FP32, tag="big3", name="o_ps")
                for i in range(N_CORES):
                    for nh in range(2):
                        nc.tensor.matmul(
                            o_ps[:, nh * 512 : (nh + 1) * 512],
                            lhsT=o_sb[:, i * TPC + m * 128 : i * TPC + (m + 1) * 128],
                            rhs=wo_sb[:, i * D + nh * 512 : i * D + nh * 512 + 512],
                            start=(i == 0),
                            stop=(i == N_CORES - 1),
                        )
                out_t = fin.tile([128, D], FP32, tag="outt", name="out_t")
                nc.vector.tensor_add(out_t[:], o_ps[:, 0:1024], bias_sb[:])
                nc.sync.dma_start(out[m * 128 : (m + 1) * 128, :], out_t[:])

    nc.compile()
    return nc


_NC_CACHE = None


def _get_nc():
    global _NC_CACHE
    if _NC_CACHE is None:
        _NC_CACHE = build_nc()
    return _NC_CACHE


def make_in_maps(x, w_qkv, w_out, b_out):
    x = np.asarray(x, dtype=np.float32)
    w_qkv = np.asarray(w_qkv, dtype=np.float32)
    w_out = np.asarray(w_out, dtype=np.float32)
    b_out = np.asarray(b_out, dtype=np.float32)

    xt_np = np.ascontiguousarray(x.reshape(T, D).T).astype(ml_dtypes.bfloat16)
    wo_np = np.ascontiguousarray(w_out.T).astype(ml_dtypes.bfloat16)
    b_np = np.ascontiguousarray(b_out.reshape(1, D))

    in_maps = []
    for c in range(N_CORES):
        rows = []
        for sec in range(3):  # q, k, v sections of w_qkv
            for hh in range(HL):
                h = HL * c + hh
                rows.append(w_qkv[sec * D + h * HD : sec * D + (h + 1) * HD, :])
        wt_np = np.ascontiguousarray(np.concatenate(rows, 0).T).astype(
            ml_dtypes.bfloat16
        )  # (1024, 384)
        in_maps.append({"xt": xt_np, "wt": wt_np, "wo": wo_np, "bias": b_np})
    return in_maps


def kernel(x, w_qkv, w_out, b_out, _trace=False, _tmpdir=None):
    in_maps = make_in_maps(x, w_qkv, w_out, b_out)
    nc = _get_nc()
    res = bass_utils.run_bass_kernel_spmd(
        nc, in_maps, core_ids=list(range(N_CORES)), trace=_trace, tmpdir=_tmpdir
    )
    out = np.concatenate([res.results[c]["out"] for c in range(N_CORES)], 0)
    kernel.last_result = res
    return out.reshape(B, NTOK, D).astype(np.float32)


# revision 9
# speedup vs baseline: 1.0417x; 1.0417x over previous
"""Distributed multi-head attention kernel for 8 TRN2 NeuronCores.

Problem: x(4,2048,1024) -> qkv proj (w_qkv 3072x1024) -> 16-head attention
(head_dim 64) -> out proj (w_out 1024x1024 + b_out).

Sharding: head-parallel. Core c owns heads {2c, 2c+1}: it computes Q/K/V for
those heads over all 8192 tokens, runs attention, then an AllToAll converts
the head-sharded attention output into a token-sharded layout for the output
projection. No all-reduce needed.

The work is issued per batch so phases overlap: QKV(b+1) runs on the
TensorEngine while attention(b)'s softmax saturates the ScalarEngine, and
each batch's AllToAll + output projection overlap the next batch's attention.

Per-core dataflow for batch b:
  1. QKV: Y^T = Wc @ X^T on PE (bf16, f32 accumulate). Q^T/K^T kept bf16 in
     SBUF ([128 = 2 heads x 64 dims, tokens]); V in natural token-major bf16
     with a ones-column appended (65 wide) per 128-token chunk (the V
     matmuls keep X as the stationary operand). PSUM->SBUF epilogues on
     VectorE (scale 1/8 folded into Q) to keep ScalarE free for exp.
  2. Attention per (head, 512-wide q-tile): S^T tiles [128 k, 512 q] on PE;
     exp on ScalarE (PSUM->SBUF bf16, batched 1536 wide); P.V on PE with
     lhsT = V-chunk [128, 65] -- the 65th output row accumulates the softmax
     denominators. A VectorE copy releases the PV PSUM bank immediately;
     normalization (reciprocal_approx_fast + partition_broadcast + multiply)
     runs off the critical path; O^T bf16 lands in the A2A buffer.
  3. AllToAll over this batch's token chunks (512KB bf16).
  4. Out proj: out = O^T.T @ w_out^T + b_out per 128-token tile (bf16 PE,
     f32 accumulate), bias added on VectorE, DMA to the core's output slice.
"""

import numpy as np
import ml_dtypes

import concourse.bass as bass
import concourse.mybir as mybir
import concourse.tile as tile
from concourse import bacc, bass_utils
from concourse.tile import add_dep_helper

FP32 = mybir.dt.float32
BF16 = mybir.dt.bfloat16
AF = mybir.ActivationFunctionType

N_CORES = 8
B, NTOK, D = 4, 2048, 1024
T = B * NTOK  # 8192 tokens total
NH, HD = 16, 64
HL = NH // N_CORES  # 2 heads per core
SCALE = float(HD) ** -0.5  # 0.125
TN = 512  # token tile for QKV / q tile for attention
NT = T // TN  # 16
KC = D // 128  # 8 contraction chunks for projections
KT = NTOK // 128  # 16 k-chunks per batch in attention
TPB = NTOK // N_CORES  # 256 tokens per (core, batch) after A2A
TPC = T // N_CORES  # 1024 tokens per core total
WCOLS = 3 * HL * HD  # 384 qkv output dims per core


def build_nc(debug=False):
    nc = bacc.Bacc(
        "TRN2", target_bir_lowering=False, debug=False, num_devices=N_CORES
    )
    xt = nc.dram_tensor("xt", [D, T], BF16, kind="ExternalInput").ap()
    wt = nc.dram_tensor("wt", [D, WCOLS], BF16, kind="ExternalInput").ap()
    wo = nc.dram_tensor("wo", [D, D], BF16, kind="ExternalInput").ap()
    bias = nc.dram_tensor("bias", [1, D], FP32, kind="ExternalInput").ap()
    # row r of out = batch r//TPB, token (core * TPB + r % TPB) of that batch
    out = nc.dram_tensor("out", [TPC, D], FP32, kind="ExternalOutput").ap()
    if debug:
        qdump = nc.dram_tensor("qdump", [128, T], FP32, kind="ExternalOutput").ap()
        kdump = nc.dram_tensor("kdump", [128, T], FP32, kind="ExternalOutput").ap()
        vdump = nc.dram_tensor(
            "vdump", [128, (T // 128) * HL * 65], FP32, kind="ExternalOutput"
        ).ap()
        adump = nc.dram_tensor(
            "adump", [N_CORES, HL * HD, TPB], FP32, kind="ExternalOutput"
        ).ap()

    with tile.TileContext(nc) as tc:
        with (
            tc.tile_pool(name="const", bufs=1) as const,
            tc.tile_pool(name="xin", bufs=10) as xin,
            tc.tile_pool(name="probs", bufs=3) as probs,
            tc.tile_pool(name="norm", bufs=3) as norm,
            tc.tile_pool(name="ot", bufs=3) as otp,
            tc.tile_pool(name="osb", bufs=2) as osbp,
            tc.tile_pool(name="fin", bufs=2) as fin,
            tc.tile_pool(name="psum", bufs=2, space="PSUM") as psum,
            tc.tile_pool(name="dram", bufs=1, space="DRAM") as dram,
        ):
            # ---- persistent SBUF state ----
            w_sb = const.tile([128, KC * WCOLS], BF16)
            nc.sync.dma_start(
                w_sb[:].rearrange("p (kc j) -> p kc j", kc=KC),
                wt.rearrange("(kc p) j -> p kc j", p=128),
            )
            wo_sb = const.tile([128, KC * D], BF16)
            nc.sync.dma_start(
                wo_sb[:].rearrange("p (kc n) -> p kc n", kc=KC),
                wo.rearrange("(kc p) n -> p kc n", p=128),
            )
            b_row = const.tile([1, D], FP32)
            nc.sync.dma_start(b_row[:], bias[:])
            bias_sb = const.tile([128, D], FP32)
            nc.gpsimd.partition_broadcast(bias_sb[:], b_row[:])

            q_sb = const.tile([128, T], BF16)  # [2 heads x 64, tokens] scaled
            k_sb = const.tile([128, T], BF16)
            # V token-major: [128 tok-in-chunk, (global chunk, head) x 65]
            v_sb = const.tile([128, (T // 128) * HL * 65], BF16)
            v3 = v_sb[:].rearrange("p (blk e) -> p blk e", e=65)
            nc.vector.memset(v3[:, :, 64:65], 1.0)

            a2a_in = []
            a2a_out = []
            for b in range(B):
                ai = dram.tile(
                    [N_CORES, HL * HD, TPB], BF16, name=f"a2a_in{b}"
                )
                ao = dram.tile(
                    [N_CORES, HL * HD, TPB], BF16, name=f"a2a_out{b}"
                )
                a2a_in.append(ai)
                a2a_out.append(ao)

            for b in range(B):
                # ---- QKV projection for this batch's 4 token tiles ----
                for t in range(4 * b, 4 * b + 4):
                    y_ps = psum.tile([128, 1536], FP32, tag="big3", name="y_ps")
                    xts = []
                    for kc in range(KC):
                        x_t = xin.tile([128, TN], BF16, tag="xt", name="x_t")
                        nc.sync.dma_start(
                            x_t[:],
                            xt[kc * 128 : (kc + 1) * 128, t * TN : (t + 1) * TN],
                        )
                        xts.append(x_t)
                    for kc in range(KC):
                        st, sp = kc == 0, kc == KC - 1
                        for m in range(2):  # Q then K, transposed layout
                            nc.tensor.matmul(
                                y_ps[:, m * 512 : (m + 1) * 512],
                                lhsT=w_sb[
                                    :,
                                    kc * WCOLS + m * 128 : kc * WCOLS + (m + 1) * 128,
                                ],
                                rhs=xts[kc][:],
                                start=st,
                                stop=sp,
                            )
                    # V natural layout: 4 token subtiles share one PSUM bank;
                    # start=True clears has_written flags bank-wide, so chain
                    # ordering deps so each accumulation group finishes before
                    # the next begins.
                    prev = None
                    for s in range(4):
                        for kc in range(KC):
                            st, sp = kc == 0, kc == KC - 1
                            mm = nc.tensor.matmul(
                                y_ps[:, 1024 + s * 128 : 1024 + (s + 1) * 128],
                                lhsT=xts[kc][:, s * 128 : (s + 1) * 128],
                                rhs=w_sb[:, kc * WCOLS + 256 : kc * WCOLS + WCOLS],
                                start=st,
                                stop=sp,
                            )
                            if prev is not None:
                                add_dep_helper(
                                    mm.ins, prev.ins, sync=False,
                                    reason="bank flag-clear order",
                                )
                            prev = mm
                    # epilogues on VectorE (keep ScalarE free for exp)
                    nc.vector.tensor_scalar_mul(
                        q_sb[:, t * TN : (t + 1) * TN], y_ps[:, 0:512], SCALE
                    )
                    nc.vector.tensor_copy(
                        k_sb[:, t * TN : (t + 1) * TN], y_ps[:, 512:1024]
                    )
                    nc.vector.tensor_copy(
                        v3[:, (t * 4) * HL : (t * 4 + 4) * HL, 0:64],
                        y_ps[:, 1024:1536]
                        .rearrange("p (s hd) -> p s hd", s=4)
                        .rearrange("p s (h d) -> p (s h) d", h=HL),
                    )

                # ---- attention for this batch ----
                for qt in range(NTOK // TN):
                    q_off = b * NTOK + qt * TN
                    pv = [
                        psum.tile([128, 512], FP32, tag="pv", name=f"pv{h}")
                        for h in range(HL)
                    ]
                    slot_list = [(kc, h) for kc in range(KT) for h in range(HL)]
                    for g0 in range(0, len(slot_list), 3):
                        group = slot_list[g0 : g0 + 3]
                        width = len(group) * 512
                        s_t = psum.tile([128, 1536], FP32, tag="big3", name="s_t")
                        for i, (kc, h) in enumerate(group):
                            nc.tensor.matmul(
                                s_t[:, i * 512 : (i + 1) * 512],
                                lhsT=k_sb[
                                    h * 64 : (h + 1) * 64,
                                    b * NTOK + kc * 128 : b * NTOK + (kc + 1) * 128,
                                ],
                                rhs=q_sb[h * 64 : (h + 1) * 64, q_off : q_off + TN],
                                start=True,
                                stop=True,
                            )
                        p_t = probs.tile([128, 1536], BF16, tag="p", name="p_t")
                        nc.scalar.activation(p_t[:, 0:width], s_t[:, 0:width], AF.Exp)
                        for i, (kc, h) in enumerate(group):
                            gc = b * KT + kc
                            nc.tensor.matmul(
                                pv[h][0:65, :],
                                lhsT=v3[:, gc * HL + h, :],
                                rhs=p_t[:, i * 512 : (i + 1) * 512],
                                start=(kc == 0),
                                stop=(kc == KT - 1),
                            )
                    for h in range(HL):
                        # single copy releases the PV PSUM bank; the rest of
                        # the normalize chain runs on SBUF off the fast path
                        o_c = norm.tile([65, 512], FP32, tag="oc", name="o_c")
                        nc.vector.tensor_copy(o_c[:], pv[h][0:65, :])
                        rec = norm.tile([1, 512], FP32, tag="rec", name="rec")
                        nc.vector.reciprocal(rec[:], o_c[64:65, :])
                        bc = norm.tile([64, 512], FP32, tag="bc", name="bc")
                        nc.gpsimd.partition_broadcast(bc[:], rec[:])
                        o_t = otp.tile([64, 512], BF16, tag="o", name="o_t")
                        nc.vector.tensor_mul(o_t[:], o_c[0:64, :], bc[:])
                        for half in range(2):
                            nc.sync.dma_start(
                                a2a_in[b][
                                    2 * qt + half, h * 64 : (h + 1) * 64, :
                                ],
                                o_t[:, half * TPB : (half + 1) * TPB],
                            )

                # ---- AllToAll over this batch's token chunks ----
                nc.gpsimd.collective_compute(
                    "AllToAll",
                    mybir.AluOpType.bypass,
                    replica_groups=[list(range(N_CORES))],
                    ins=[a2a_in[b].opt()],
                    outs=[a2a_out[b].opt()],
                )

                # ---- output projection for this batch's tokens ----
                o_sb = osbp.tile([128, N_CORES * TPB], BF16, tag="osb", name="o_sb")
                for i in range(N_CORES):
                    nc.sync.dma_start(
                        o_sb[:, i * TPB : (i + 1) * TPB], a2a_out[b][i, :, :]
                    )
                for m in range(TPB // 128):
                    o_ps = psum.tile([128, 1536], FP32, tag="big3", name="o_ps")
                    for i in range(N_CORES):
                        for nh in range(2):
                            nc.tensor.matmul(
                                o_ps[:, nh * 512 : (nh + 1) * 512],
                                lhsT=o_sb[
                                    :, i * TPB + m * 128 : i * TPB + (m + 1) * 128
                                ],
                                rhs=wo_sb[:, i * D + nh * 512 : i * D + nh * 512 + 512],
                                start=(i == 0),
                                stop=(i == N_CORES - 1),
                            )
                    out_t = fin.tile([128, D], FP32, tag="outt", name="out_t")
                    nc.vector.tensor_add(out_t[:], o_ps[:, 0:1024], bias_sb[:])
                    nc.sync.dma_start(
                        out[b * TPB + m * 128 : b * TPB + (m + 1) * 128, :],
                        out_t[:],
                    )

            if debug:
                for t in range(NT):
                    d1 = fin.tile([128, TN], FP32, tag="outt", name="d1")
                    nc.vector.tensor_copy(d1[:], q_sb[:, t * TN : (t + 1) * TN])
                    nc.sync.dma_start(qdump[:, t * TN : (t + 1) * TN], d1[:])
                    d2 = fin.tile([128, TN], FP32, tag="outt", name="d2")
                    nc.vector.tensor_copy(d2[:], k_sb[:, t * TN : (t + 1) * TN])
                    nc.sync.dma_start(kdump[:, t * TN : (t + 1) * TN], d2[:])
                nv = (T // 128) * HL * 65
                for j in range(0, nv, 1024):
                    wdt = min(1024, nv - j)
                    d3 = fin.tile([128, 1024], FP32, tag="outt", name="d3")
                    nc.vector.tensor_copy(d3[:, 0:wdt], v_sb[:, j : j + wdt])
                    nc.sync.dma_start(vdump[:, j : j + wdt], d3[:, 0:wdt])
                for i in range(N_CORES):
                    d4 = fin.tile([128, TPB], BF16, tag="outt", name="d4")
                    nc.sync.dma_start(d4[:], a2a_in[0][i, :, :])
                    d5 = fin.tile([128, TPB], FP32, tag="outt", name="d5")
                    nc.vector.tensor_copy(d5[:], d4[:])
                    nc.sync.dma_start(adump[i, :, :], d5[:])

    nc.compile()
    return nc


_NC_CACHE = None


def _get_nc():
    global _NC_CACHE
    if _NC_CACHE is None:
        _NC_CACHE = build_nc()
    return _NC_CACHE


def make_in_maps(x, w_qkv, w_out, b_out):
    x = np.asarray(x, dtype=np.float32)
    w_qkv = np.asarray(w_qkv, dtype=np.float32)
    w_out = np.asarray(w_out, dtype=np.float32)
    b_out = np.asarray(b_out, dtype=np.float32)

    xt_np = np.ascontiguousarray(x.reshape(T, D).T).astype(ml_dtypes.bfloat16)
    wo_np = np.ascontiguousarray(w_out.T).astype(ml_dtypes.bfloat16)
    b_np = np.ascontiguousarray(b_out.reshape(1, D))

    in_maps = []
    for c in range(N_CORES):
        rows = []
        for sec in range(3):  # q, k, v sections of w_qkv
            for hh in range(HL):
                h = HL * c + hh
                rows.append(w_qkv[sec * D + h * HD : sec * D + (h + 1) * HD, :])
        wt_np = np.ascontiguousarray(np.concatenate(rows, 0).T).astype(
            ml_dtypes.bfloat16
        )  # (1024, 384)
        in_maps.append({"xt": xt_np, "wt": wt_np, "wo": wo_np, "bias": b_np})
    return in_maps


def kernel(x, w_qkv, w_out, b_out, _trace=False, _tmpdir=None):
    in_maps = make_in_maps(x, w_qkv, w_out, b_out)
    nc = _get_nc()
    res = bass_utils.run_bass_kernel_spmd(
        nc, in_maps, core_ids=list(range(N_CORES)), trace=_trace, tmpdir=_tmpdir
    )
    # core j's out rows: [b*TPB + u] = batch b, token b*NTOK + j*TPB + u
    stacked = np.stack([res.results[c]["out"] for c in range(N_CORES)], 0)
    full = stacked.reshape(N_CORES, B, TPB, D).transpose(1, 0, 2, 3)
    kernel.last_result = res
    return np.ascontiguousarray(full.reshape(B, NTOK, D)).astype(np.float32)


# revision 10
# speedup vs baseline: 1.2152x; 1.1665x over previous
"""Distributed multi-head attention kernel for 8 TRN2 NeuronCores.

Problem: x(4,2048,1024) -> qkv proj (w_qkv 3072x1024) -> 16-head attention
(head_dim 64) -> out proj (w_out 1024x1024 + b_out).

Sharding: head-parallel. Core c owns heads {2c, 2c+1}: it computes Q/K/V for
those heads over all 8192 tokens, runs attention, then an AllToAll converts
the head-sharded attention output into a token-sharded layout for the output
projection. No all-reduce needed.

The work is issued per batch so phases overlap: QKV(b+1) runs on the
TensorEngine while attention(b)'s softmax saturates the ScalarEngine, and
each batch's AllToAll + output projection overlap the next batch's attention.

Per-core dataflow for batch b:
  1. QKV: Y^T = Wc @ X^T on PE (bf16, f32 accumulate). Q^T/K^T kept bf16 in
     SBUF ([128 = 2 heads x 64 dims, tokens]); V in natural token-major bf16
     with a ones-column appended (65 wide) per 128-token chunk (the V
     matmuls keep X as the stationary operand). PSUM->SBUF epilogues on
     VectorE (scale 1/8 folded into Q) to keep ScalarE free for exp.
  2. Attention per (head, 512-wide q-tile): S^T tiles [128 k, 512 q] on PE;
     exp on ScalarE (PSUM->SBUF bf16, batched 1536 wide); P.V on PE with
     lhsT = V-chunk [128, 65] -- the 65th output row accumulates the softmax
     denominators. A VectorE copy releases the PV PSUM bank immediately;
     normalization (reciprocal_approx_fast + partition_broadcast + multiply)
     runs off the critical path; O^T bf16 lands in the A2A buffer.
  3. AllToAll over this batch's token chunks (512KB bf16).
  4. Out proj: out = O^T.T @ w_out^T + b_out per 128-token tile (bf16 PE,
     f32 accumulate), bias added on VectorE, DMA to the core's output slice.
"""

import numpy as np
import ml_dtypes

import concourse.bass as bass
import concourse.mybir as mybir
import concourse.tile as tile
from concourse import bacc, bass_utils
from concourse.tile import add_dep_helper

FP32 = mybir.dt.float32
BF16 = mybir.dt.bfloat16
AF = mybir.ActivationFunctionType

N_CORES = 8
B, NTOK, D = 4, 2048, 1024
T = B * NTOK  # 8192 tokens total
NH, HD = 16, 64
HL = NH // N_CORES  # 2 heads per core
SCALE = float(HD) ** -0.5  # 0.125
TN = 512  # token tile for QKV / q tile for attention
NT = T // TN  # 16
KC = D // 128  # 8 contraction chunks for projections
KT = NTOK // 128  # 16 k-chunks per batch in attention
TPB = NTOK // N_CORES  # 256 tokens per (core, batch) after A2A
TPC = T // N_CORES  # 1024 tokens per core total
WCOLS = 3 * HL * HD  # 384 qkv output dims per core


def build_nc(debug=False):
    nc = bacc.Bacc(
        "TRN2", target_bir_lowering=False, debug=False, num_devices=N_CORES
    )
    xt = nc.dram_tensor("xt", [D, T], BF16, kind="ExternalInput").ap()
    wt = nc.dram_tensor("wt", [D, WCOLS], BF16, kind="ExternalInput").ap()
    wo = nc.dram_tensor("wo", [D, D], BF16, kind="ExternalInput").ap()
    bias = nc.dram_tensor("bias", [1, D], FP32, kind="ExternalInput").ap()
    # row r of out = batch r//TPB, token (core * TPB + r % TPB) of that batch
    out = nc.dram_tensor("out", [TPC, D], FP32, kind="ExternalOutput").ap()
    if debug:
        qdump = nc.dram_tensor("qdump", [128, T], FP32, kind="ExternalOutput").ap()
        kdump = nc.dram_tensor("kdump", [128, T], FP32, kind="ExternalOutput").ap()
        vdump = nc.dram_tensor(
            "vdump", [128, (T // 128) * HL * 65], FP32, kind="ExternalOutput"
        ).ap()
        adump = nc.dram_tensor(
            "adump", [N_CORES, HL * HD, TPB], FP32, kind="ExternalOutput"
        ).ap()

    with tile.TileContext(nc) as tc:
        with (
            tc.tile_pool(name="const", bufs=1) as const,
            tc.tile_pool(name="xin", bufs=10) as xin,
            tc.tile_pool(name="probs", bufs=3) as probs,
            tc.tile_pool(name="norm", bufs=3) as norm,
            tc.tile_pool(name="ot", bufs=3) as otp,
            tc.tile_pool(name="osb", bufs=2) as osbp,
            tc.tile_pool(name="fin", bufs=2) as fin,
            tc.tile_pool(name="psum", bufs=2, space="PSUM") as psum,
            tc.tile_pool(name="dram", bufs=1, space="DRAM") as dram,
        ):
            # ---- persistent SBUF state ----
            w_sb = const.tile([128, KC * WCOLS], BF16)
            nc.sync.dma_start(
                w_sb[:].rearrange("p (kc j) -> p kc j", kc=KC),
                wt.rearrange("(kc p) j -> p kc j", p=128),
            )
            wo_sb = const.tile([128, KC * D], BF16)
            nc.sync.dma_start(
                wo_sb[:].rearrange("p (kc n) -> p kc n", kc=KC),
                wo.rearrange("(kc p) n -> p kc n", p=128),
            )
            b_row = const.tile([1, D], FP32)
            nc.sync.dma_start(b_row[:], bias[:])
            bias_sb = const.tile([128, D], FP32)
            nc.gpsimd.partition_broadcast(bias_sb[:], b_row[:])

            q_sb = const.tile([128, T], BF16)  # [2 heads x 64, tokens] scaled
            k_sb = const.tile([128, T], BF16)
            # V token-major: [128 tok-in-chunk, (global chunk, head) x 65]
            v_sb = const.tile([128, (T // 128) * HL * 65], BF16)
            v3 = v_sb[:].rearrange("p (blk e) -> p blk e", e=65)
            nc.vector.memset(v3[:, :, 64:65], 1.0)

            a2a_in = []
            a2a_out = []
            for b in range(B):
                ai = dram.tile(
                    [N_CORES, HL * HD, TPB], BF16, name=f"a2a_in{b}"
                )
                ao = dram.tile(
                    [N_CORES, HL * HD, TPB], BF16, name=f"a2a_out{b}"
                )
                a2a_in.append(ai)
                a2a_out.append(ao)

            for b in range(B):
                # ---- QKV projection for this batch's 4 token tiles ----
                for t in range(4 * b, 4 * b + 4):
                    y_ps = psum.tile([128, 1536], FP32, tag="big3", name="y_ps")
                    xts = []
                    for kc in range(KC):
                        x_t = xin.tile([128, TN], BF16, tag="xt", name="x_t")
                        nc.sync.dma_start(
                            x_t[:],
                            xt[kc * 128 : (kc + 1) * 128, t * TN : (t + 1) * TN],
                        )
                        xts.append(x_t)
                    for kc in range(KC):
                        st, sp = kc == 0, kc == KC - 1
                        for m in range(2):  # Q then K, transposed layout
                            nc.tensor.matmul(
                                y_ps[:, m * 512 : (m + 1) * 512],
                                lhsT=w_sb[
                                    :,
                                    kc * WCOLS + m * 128 : kc * WCOLS + (m + 1) * 128,
                                ],
                                rhs=xts[kc][:],
                                start=st,
                                stop=sp,
                            )
                    # V natural layout: 4 token subtiles share one PSUM bank;
                    # start=True clears has_written flags bank-wide, so chain
                    # ordering deps so each accumulation group finishes before
                    # the next begins.
                    prev = None
                    for s in range(4):
                        for kc in range(KC):
                            st, sp = kc == 0, kc == KC - 1
                            mm = nc.tensor.matmul(
                                y_ps[:, 1024 + s * 128 : 1024 + (s + 1) * 128],
                                lhsT=xts[kc][:, s * 128 : (s + 1) * 128],
                                rhs=w_sb[:, kc * WCOLS + 256 : kc * WCOLS + WCOLS],
                                start=st,
                                stop=sp,
                            )
                            if prev is not None:
                                add_dep_helper(
                                    mm.ins, prev.ins, sync=False,
                                    reason="bank flag-clear order",
                                )
                            prev = mm
                    # epilogues on VectorE (keep ScalarE free for exp)
                    nc.vector.tensor_scalar_mul(
                        q_sb[:, t * TN : (t + 1) * TN], y_ps[:, 0:512], SCALE
                    )
                    nc.vector.tensor_copy(
                        k_sb[:, t * TN : (t + 1) * TN], y_ps[:, 512:1024]
                    )
                    nc.vector.tensor_copy(
                        v3[:, (t * 4) * HL : (t * 4 + 4) * HL, 0:64],
                        y_ps[:, 1024:1536]
                        .rearrange("p (s hd) -> p s hd", s=4)
                        .rearrange("p s (h d) -> p (s h) d", h=HL),
                    )

                # ---- attention for this batch ----
                for qt in range(NTOK // TN):
                    q_off = b * NTOK + qt * TN
                    pv = [
                        psum.tile([128, 512], FP32, tag="pv", name=f"pv{h}")
                        for h in range(HL)
                    ]
                    slot_list = [(kc, h) for kc in range(KT) for h in range(HL)]
                    for g0 in range(0, len(slot_list), 3):
                        group = slot_list[g0 : g0 + 3]
                        width = len(group) * 512
                        s_t = psum.tile([128, 1536], FP32, tag="big3", name="s_t")
                        for i, (kc, h) in enumerate(group):
                            nc.tensor.matmul(
                                s_t[:, i * 512 : (i + 1) * 512],
                                lhsT=k_sb[
                                    h * 64 : (h + 1) * 64,
                                    b * NTOK + kc * 128 : b * NTOK + (kc + 1) * 128,
                                ],
                                rhs=q_sb[h * 64 : (h + 1) * 64, q_off : q_off + TN],
                                start=True,
                                stop=True,
                            )
                        p_t = probs.tile([128, 1536], BF16, tag="p", name="p_t")
                        nc.scalar.activation(p_t[:, 0:width], s_t[:, 0:width], AF.Exp)
                        for i, (kc, h) in enumerate(group):
                            gc = b * KT + kc
                            nc.tensor.matmul(
                                pv[h][0:65, :],
                                lhsT=v3[:, gc * HL + h, :],
                                rhs=p_t[:, i * 512 : (i + 1) * 512],
                                start=(kc == 0),
                                stop=(kc == KT - 1),
                            )
                    for h in range(HL):
                        # single copy releases the PV PSUM bank; the rest of
                        # the normalize chain runs on SBUF off the fast path
                        o_c = norm.tile([65, 512], FP32, tag="oc", name="o_c")
                        nc.vector.tensor_copy(o_c[:], pv[h][0:65, :])
                        rec = norm.tile([1, 512], FP32, tag="rec", name="rec")
                        nc.vector.reciprocal(rec[:], o_c[64:65, :])
                        bc = norm.tile([64, 512], FP32, tag="bc", name="bc")
                        nc.gpsimd.partition_broadcast(bc[:], rec[:])
                        o_t = otp.tile([64, 512], BF16, tag="o", name="o_t")
                        nc.vector.tensor_mul(o_t[:], o_c[0:64, :], bc[:])
                        for half in range(2):
                            nc.sync.dma_start(
                                a2a_in[b][
                                    2 * qt + half, h * 64 : (h + 1) * 64, :
                                ],
                                o_t[:, half * TPB : (half + 1) * TPB],
                            )

                # ---- AllToAll over this batch's token chunks ----
                nc.gpsimd.collective_compute(
                    "AllToAll",
                    mybir.AluOpType.bypass,
                    replica_groups=[list(range(N_CORES))],
                    ins=[a2a_in[b].opt()],
                    outs=[a2a_out[b].opt()],
                )

                # ---- output projection for this batch's tokens ----
                o_sb = osbp.tile([128, N_CORES * TPB], BF16, tag="osb", name="o_sb")
                for i in range(N_CORES):
                    nc.sync.dma_start(
                        o_sb[:, i * TPB : (i + 1) * TPB], a2a_out[b][i, :, :]
                    )
                for m in range(TPB // 128):
                    # pv-tag PSUM tiles so the big3 slot queue (QKV/S tiles of
                    # the next batch) is not blocked behind the AllToAll
                    o_ps = [
                        psum.tile([128, 512], FP32, tag="pv", name=f"o_ps{nh}")
                        for nh in range(2)
                    ]
                    for i in range(N_CORES):
                        for nh in range(2):
                            nc.tensor.matmul(
                                o_ps[nh][:, :],
                                lhsT=o_sb[
                                    :, i * TPB + m * 128 : i * TPB + (m + 1) * 128
                                ],
                                rhs=wo_sb[:, i * D + nh * 512 : i * D + nh * 512 + 512],
                                start=(i == 0),
                                stop=(i == N_CORES - 1),
                            )
                    out_t = fin.tile([128, D], FP32, tag="outt", name="out_t")
                    for nh in range(2):
                        nc.vector.tensor_add(
                            out_t[:, nh * 512 : (nh + 1) * 512],
                            o_ps[nh][:, :],
                            bias_sb[:, nh * 512 : (nh + 1) * 512],
                        )
                    nc.sync.dma_start(
                        out[b * TPB + m * 128 : b * TPB + (m + 1) * 128, :],
                        out_t[:],
                    )

            if debug:
                for t in range(NT):
                    d1 = fin.tile([128, TN], FP32, tag="outt", name="d1")
                    nc.vector.tensor_copy(d1[:], q_sb[:, t * TN : (t + 1) * TN])
                    nc.sync.dma_start(qdump[:, t * TN : (t + 1) * TN], d1[:])
                    d2 = fin.tile([128, TN], FP32, tag="outt", name="d2")
                    nc.vector.tensor_copy(d2[:], k_sb[:, t * TN : (t + 1) * TN])
                    nc.sync.dma_start(kdump[:, t * TN : (t + 1) * TN], d2[:])
                nv = (T // 128) * HL * 65
                for j in range(0, nv, 1024):
                    wdt = min(1024, nv - j)
                    d3 = fin.tile([128, 1024], FP32, tag="outt", name="d3")
                    nc.vector.tensor_copy(d3[:, 0:wdt], v_sb[:, j : j + wdt])
                    nc.sync.dma_start(vdump[:, j : j + wdt], d3[:, 0:wdt])
                for i in range(N_CORES):
                    d4 = fin.tile([128, TPB], BF16, tag="outt", name="d4")
                    nc.sync.dma_start(d4[:], a2a_in[0][i, :, :])
                    d5 = fin.tile([128, TPB], FP32, tag="outt", name="d5")
                    nc.vector.tensor_copy(d5[:], d4[:])
                    nc.sync.dma_start(adump[i, :, :], d5[:])

    nc.compile()
    return nc


_NC_CACHE = None


def _get_nc():
    global _NC_CACHE
    if _NC_CACHE is None:
        _NC_CACHE = build_nc()
    return _NC_CACHE


def make_in_maps(x, w_qkv, w_out, b_out):
    x = np.asarray(x, dtype=np.float32)
    w_qkv = np.asarray(w_qkv, dtype=np.float32)
    w_out = np.asarray(w_out, dtype=np.float32)
    b_out = np.asarray(b_out, dtype=np.float32)

    xt_np = np.ascontiguousarray(x.reshape(T, D).T).astype(ml_dtypes.bfloat16)
    wo_np = np.ascontiguousarray(w_out.T).astype(ml_dtypes.bfloat16)
    b_np = np.ascontiguousarray(b_out.reshape(1, D))

    in_maps = []
    for c in range(N_CORES):
        rows = []
        for sec in range(3):  # q, k, v sections of w_qkv
            for hh in range(HL):
                h = HL * c + hh
                rows.append(w_qkv[sec * D + h * HD : sec * D + (h + 1) * HD, :])
        wt_np = np.ascontiguousarray(np.concatenate(rows, 0).T).astype(
            ml_dtypes.bfloat16
        )  # (1024, 384)
        in_maps.append({"xt": xt_np, "wt": wt_np, "wo": wo_np, "bias": b_np})
    return in_maps


def kernel(x, w_qkv, w_out, b_out, _trace=False, _tmpdir=None):
    in_maps = make_in_maps(x, w_qkv, w_out, b_out)
    nc = _get_nc()
    res = bass_utils.run_bass_kernel_spmd(
        nc, in_maps, core_ids=list(range(N_CORES)), trace=_trace, tmpdir=_tmpdir
    )
    # core j's out rows: [b*TPB + u] = batch b, token b*NTOK + j*TPB + u
    stacked = np.stack([res.results[c]["out"] for c in range(N_CORES)], 0)
    full = stacked.reshape(N_CORES, B, TPB, D).transpose(1, 0, 2, 3)
    kernel.last_result = res
    return np.ascontiguousarray(full.reshape(B, NTOK, D)).astype(np.float32)
